# revision 22
# baseline (speedup 1.0000x reference)
"""Trainium2 Bass kernel for CurveChannel: piecewise-linear per-channel curve
+ 1x1 conv (C->1) + hardtanh(0,1).

out[b,0,h,w] = clip( sum_{p,c} W[p,c] * relu(x[b,c,h,w] - shift[p,c]) + conv_b,
                     0, 1 )         where W[p,c] = slopes[p,c] * conv_w[c]

Sharding: pure data parallel over batch (8 images -> 8 cores). Params are tiny
and get folded host-side into per-(p,c) weights; zero-weight terms contribute
exactly 0 and are skipped.

Per-core structure (memory-bound; ~4 MiB HBM traffic/core is the floor):
  - chunk the flat spatial dim; one combined HWDGE in-DMA per chunk
  - per nonzero term, a weighted relu into a slice of a per-chunk wide tile:
    ScalarE activation for most terms (W>0: W*relu(x-s) == relu(W*x - W*s);
    W<0: W*relu(x-s) == -relu(-W*x + W*s), subtracted later), with one
    shift==0 term offloaded to VectorE to balance engine load
  - VectorE combines slices (tensor-tensor adds for few terms, a strided
    tensor_reduce for many) and clips; per-chunk out-DMA
  - the last chunks are smaller to shorten the serial tail
"""

import os

import numpy as np

import concourse.bacc as bacc
import concourse.bass as bass
import concourse.mybir as mybir
import concourse.tile as tile
from concourse.bass_utils import run_bass_kernel_spmd

N_CORES = 8
C_IN = 3
H = 512
W_IMG = 512
P = 128                      # SBUF partitions
SPATIAL = H * W_IMG          # 262144
FREE = SPATIAL // P          # 2048 fp32 per partition per channel

# chunk schedule over the free dim (sums to FREE); smaller final chunks
# shorten the compute+store tail that cannot overlap the DMA stream
SCHEDULE = [256] * 7 + [128, 128]

F32 = mybir.dt.float32

LAST_RESULTS = None          # BassKernelResults of the most recent run (for test.py)


def _build_nc(terms, bias, reps=1, schedule=None, bufs=8, dve_offload=True,
              out_engine="sync"):
    """terms: list of (channel, weight, shift) with weight != 0.

    reps > 1 unrolls the whole pass multiple times over the same data --
    only used for benchmarking (marginal time per pass = device time with
    host/RPC constants cancelled).
    """
    schedule = list(schedule or SCHEDULE)
    assert sum(schedule) == FREE
    nc = bacc.Bacc(trn_type="TRN2", debug=False)
    x_t = nc.dram_tensor("x", [C_IN, P, FREE], F32, kind="ExternalInput")
    out_t = nc.dram_tensor("out", [P, FREE], F32, kind="ExternalOutput")

    pos = [(c, w, s) for c, w, s in terms if w > 0]
    neg = [(c, w, s) for c, w, s in terms if w < 0]
    # offload one positive shift==0 term to the vector engine (one
    # tensor_scalar: (x max 0) mult w) when ScalarE would otherwise have more
    # per-chunk work than VectorE; consumed last so the combine chain stays
    # same-engine
    dve_term = None
    if dve_offload and len(pos) + len(neg) >= 3:
        for i, (c, w, s) in enumerate(pos):
            if s == 0.0:
                dve_term = pos.pop(i)
                break
    ordered = pos + neg
    used_channels = sorted({c for c, _, _ in terms})
    cidx = {c: i for i, c in enumerate(used_channels)}
    nch = len(used_channels)
    nt = len(ordered)            # ACT-written slice count
    npos = len(pos)

    with tile.TileContext(nc) as tc:
        with (
            tc.tile_pool(name="xin", bufs=bufs) as xpool,
            tc.tile_pool(name="work", bufs=bufs) as wpool,
            tc.tile_pool(name="out", bufs=bufs) as opool,
        ):
          for _ in range(reps):
            off = 0
            for CH in schedule:
                cs = slice(off, off + CH)
                off += CH
                res = opool.tile([P, CH], F32, tag="res")
                if nt == 0 and dve_term is None:
                    nc.vector.memset(res[:], float(np.clip(bias, 0.0, 1.0)))
                    nc.sync.dma_start(out=out_t[:, cs], in_=res[:])
                    continue

                xt = xpool.tile([P, nch * CH], F32, tag="x")
                if nch == C_IN:
                    nc.sync.dma_start(
                        out=xt[:],
                        in_=x_t[:, :, cs].rearrange("c p f -> p c f"),
                    )
                else:
                    for c in used_channels:
                        nc.sync.dma_start(
                            out=xt[:, bass.ts(cidx[c], CH)],
                            in_=x_t[c, :, cs],
                        )

                nslices = nt + (1 if dve_term is not None else 0)
                wide = wpool.tile([P, nslices * CH], F32, tag="wide")
                for i, (c, w, s) in enumerate(ordered):
                    sl = wide[:, bass.ts(i, CH)]
                    xs = xt[:, bass.ts(cidx[c], CH)]
                    if w > 0:
                        nc.scalar.activation(
                            sl, xs, mybir.ActivationFunctionType.Relu,
                            bias=-w * s, scale=w,
                        )
                    else:
                        nc.scalar.activation(
                            sl, xs, mybir.ActivationFunctionType.Relu,
                            bias=w * s, scale=-w,
                        )
                if dve_term is not None:
                    c, w, s = dve_term
                    nc.vector.tensor_scalar(
                        wide[:, bass.ts(nslices - 1, CH)],
                        xt[:, bass.ts(cidx[c], CH)],
                        0.0, w, mybir.AluOpType.max, mybir.AluOpType.mult,
                    )

                def combine(idxs, tag):
                    """sum of the given wide slices -> AP (None if empty)"""
                    if not idxs:
                        return None
                    if len(idxs) == 1:
                        return wide[:, bass.ts(idxs[0], CH)]
                    if len(idxs) <= 4 and idxs == list(
                        range(idxs[0], idxs[0] + len(idxs))
                    ):
                        acc = wpool.tile([P, CH], F32, tag=tag)
                        nc.vector.tensor_add(
                            acc[:], wide[:, bass.ts(idxs[0], CH)],
                            wide[:, bass.ts(idxs[1], CH)],
                        )
                        for k in idxs[2:]:
                            nc.vector.tensor_add(
                                acc[:], acc[:], wide[:, bass.ts(k, CH)]
                            )
                        return acc[:]
                    lo, hi = idxs[0], idxs[-1] + 1
                    dst = wpool.tile([P, CH], F32, tag=tag)
                    v = wide[:, lo * CH:hi * CH].rearrange(
                        "p (c f) -> p f c", c=hi - lo
                    )
                    nc.vector.tensor_reduce(
                        dst[:], v, axis=mybir.AxisListType.X,
                        op=mybir.AluOpType.add,
                    )
                    return dst[:]

                pos_idx = list(range(npos)) + (
                    [nslices - 1] if dve_term is not None else []
                )
                # keep the DVE slice in the positive combine only via the add
                # chain (it's not contiguous with the ACT positive slices)
                if dve_term is not None and npos >= 1:
                    rp_part = combine(list(range(npos)), "redp")
                    acc = wpool.tile([P, CH], F32, tag="accp")
                    nc.vector.tensor_add(
                        acc[:], rp_part, wide[:, bass.ts(nslices - 1, CH)]
                    )
                    rp = acc[:]
                elif dve_term is not None:
                    rp = wide[:, bass.ts(nslices - 1, CH)]
                else:
                    rp = combine(list(range(npos)), "redp")
                rn = combine(list(range(npos, nt)), "redn")

                if rp is not None and rn is not None:
                    comb = wpool.tile([P, CH], F32, tag="comb")
                    nc.vector.tensor_sub(comb[:], rp, rn)
                    comb = comb[:]
                elif rp is not None:
                    comb = rp
                else:
                    comb = wpool.tile([P, CH], F32, tag="comb")
                    nc.vector.tensor_scalar_mul(comb, rn, -1.0)
                    comb = comb[:]

                if bias != 0.0:
                    nc.vector.tensor_scalar(
                        res[:], comb, bias, 0.0,
                        mybir.AluOpType.add, mybir.AluOpType.max,
                    )
                    nc.vector.tensor_scalar_min(res[:], res[:], 1.0)
                else:
                    nc.vector.tensor_scalar(
                        res[:], comb, 0.0, 1.0,
                        mybir.AluOpType.max, mybir.AluOpType.min,
                    )
                oeng = nc.sync if out_engine == "sync" else nc.gpsimd
                oeng.dma_start(out=out_t[:, cs], in_=res[:])
    nc.compile()
    return nc


LINEAR_SCHEDULE = [512, 640, 512, 384]


def _build_linear_nc(w_common, bias, clip_mode, reps=1, schedule=None):
    """Raw-bacc fast path: out = clip(w_common*(x0+x1+x2) + bias, 0, 1) with
    every relu a no-op for the concrete input. Per chunk: 3 per-channel
    in-DMAs, two tensor_adds, one or two tensor_scalars, out-DMA. The first
    add is gated only on channels 0+1 so VectorE starts one DMA earlier.

    clip_mode "fused": bias==0, w>=0, x>=0 -- the lower clip is a no-op by
    f32 nonneg closure and the upper clip folds into the scale op
    ((sum mult w) min 1), which is exact. Otherwise the full two-op clip.
    """
    import contextlib
    schedule = list(schedule or LINEAR_SCHEDULE)
    assert sum(schedule) == FREE
    n = len(schedule)
    nc = bacc.Bacc(trn_type="TRN2", debug=False)
    x_t = nc.dram_tensor("x", [C_IN, P, FREE], F32, kind="ExternalInput")
    out_t = nc.dram_tensor("out", [P, FREE], F32, kind="ExternalOutput")
    xts = [nc.alloc_sbuf_tensor(f"xt{j}", [P, C_IN * CH], F32)
           for j, CH in enumerate(schedule)]
    tmps = [nc.alloc_sbuf_tensor(f"tmp{j}", [P, CH], F32)
            for j, CH in enumerate(schedule)]
    ress = [nc.alloc_sbuf_tensor(f"res{j}", [P, CH], F32)
            for j, CH in enumerate(schedule)]
    offs = np.cumsum([0] + schedule)
    with contextlib.ExitStack() as ctx:
        inA = [ctx.enter_context(nc.semaphore(f"inA{j}")) for j in range(n)]
        inB = [ctx.enter_context(nc.semaphore(f"inB{j}")) for j in range(n)]
        s1 = ctx.enter_context(nc.semaphore("s1"))
        s2 = ctx.enter_context(nc.semaphore("s2"))
        s3 = ctx.enter_context(nc.semaphore("s3"))
        dve_sem = ctx.enter_context(nc.semaphore("dve_sem"))
        out_sems = [ctx.enter_context(nc.semaphore(f"out{j}")) for j in range(n)]
        block = ctx.enter_context(nc.Block())

        @block.sync
        def _(sync):
            for r in range(reps):
                for j, CH in enumerate(schedule):
                    cs = slice(int(offs[j]), int(offs[j]) + CH)
                    if r > 0:
                        # WAR: previous rep's TT2 must have consumed xt{j}
                        sync.wait_ge(s2, (r - 1) * n + j + 1)
                    sync.dma_start(out=xts[j].ap()[:, bass.ts(0, CH)],
                                   in_=x_t[0, :, cs]).then_inc(inA[j], 16)
                    sync.dma_start(out=xts[j].ap()[:, bass.ts(1, CH)],
                                   in_=x_t[1, :, cs]).then_inc(inA[j], 16)
                    sync.dma_start(out=xts[j].ap()[:, bass.ts(2, CH)],
                                   in_=x_t[2, :, cs]).then_inc(inB[j], 16)
                for j, CH in enumerate(schedule):
                    cs = slice(int(offs[j]), int(offs[j]) + CH)
                    sync.wait_ge(dve_sem, r * n + j + 1)
                    sync.dma_start(out=out_t[:, cs],
                                   in_=ress[j].ap()).then_inc(out_sems[j], 16)
            for j in range(n):
                sync.wait_ge(out_sems[j], 16 * reps)

        @block.vector
        def _(vector):
            for r in range(reps):
                for j, CH in enumerate(schedule):
                    xa = xts[j].ap()
                    k = r * n + j + 1
                    vector.wait_ge(inA[j], 32 * (r + 1))
                    vector.tensor_add(
                        tmps[j].ap(), xa[:, bass.ts(0, CH)],
                        xa[:, bass.ts(1, CH)],
                    ).then_inc(s1, 1)
                    vector.wait_ge(inB[j], 16 * (r + 1))
                    vector.wait_ge(s1, k)
                    vector.tensor_add(
                        tmps[j].ap(), tmps[j].ap(), xa[:, bass.ts(2, CH)]
                    ).then_inc(s2, 1)
                    vector.wait_ge(s2, k)
                    if r > 0:
                        # WAR: previous rep's out-DMA must have read res{j}
                        vector.wait_ge(out_sems[j], 16 * r)
                    if clip_mode == "fused":
                        vector.tensor_scalar(
                            ress[j].ap(), tmps[j].ap(), w_common, 1.0,
                            mybir.AluOpType.mult, mybir.AluOpType.min,
                        ).then_inc(dve_sem, 1)
                    else:
                        vector.tensor_scalar(
                            ress[j].ap(), tmps[j].ap(), w_common, bias,
                            mybir.AluOpType.mult, mybir.AluOpType.add,
                        ).then_inc(s3, 1)
                        vector.wait_ge(s3, k)
                        vector.tensor_scalar(
                            ress[j].ap(), ress[j].ap(), 0.0, 1.0,
                            mybir.AluOpType.max, mybir.AluOpType.min,
                        ).then_inc(dve_sem, 1)
    nc.compile()
    return nc


_NC_CACHE = {}


def _fast_linear_plan(terms, bias, xmin):
    """If every relu is a no-op for the concrete input (all shifts <= xmin),
    the model is linear: out = clip(sum_c Wc*x_c + b', 0, 1) with
    Wc = sum_p w[p,c], b' = bias - sum w*s. Returns (w_common, b', clip_mode)
    when additionally all Wc are equal (single post-scale), else None."""
    if not terms:
        return None
    if any(s > xmin for _, _, s in terms):
        return None
    bprime = bias - sum(w * s for _, w, s in terms)
    wc = {}
    for c, w, s in terms:
        wc[c] = wc.get(c, 0.0) + w
    if set(wc) != set(range(C_IN)):
        return None
    vals = list(wc.values())
    if max(vals) != min(vals):
        return None
    w_common = vals[0]
    if bprime == 0.0 and w_common >= 0.0 and xmin >= 0.0:
        clip_mode = "fused"      # exact: see _build_linear_nc
    else:
        clip_mode = "full"
    return (w_common, bprime, clip_mode)


def kernel(x, shift, slopes, conv_w, conv_b):
    global LAST_RESULTS
    x = np.ascontiguousarray(np.asarray(x, dtype=np.float32))
    shift = np.asarray(shift, dtype=np.float32)
    slopes = np.asarray(slopes, dtype=np.float32)
    conv_w = np.asarray(conv_w, dtype=np.float32)
    conv_b = np.asarray(conv_b, dtype=np.float32)

    B = x.shape[0]
    assert x.shape == (N_CORES, C_IN, H, W_IMG), x.shape

    wmat = slopes * conv_w[None, :]                      # (npts, C)
    npts = wmat.shape[0]
    terms = tuple(
        (c, float(wmat[p, c]), float(shift[p, c]))
        for p in range(npts) for c in range(C_IN)
        if wmat[p, c] != 0.0
    )
    bias = float(conv_b.reshape(-1)[0])

    xmin = float(x.min())
    plan = _fast_linear_plan(terms, bias, xmin)
    if plan is not None:
        w_common, bprime, clip_mode = plan
        key = ("lin", w_common, bprime, clip_mode)
        nc = _NC_CACHE.get(key)
        if nc is None:
            nc = _build_linear_nc(w_common, bprime, clip_mode)
            _NC_CACHE[key] = nc
    else:
        key = (terms, bias)
        nc = _NC_CACHE.get(key)
        if nc is None:
            nc = _build_nc(terms, bias)
            _NC_CACHE[key] = nc

    xs = x.reshape(B, C_IN, P, FREE)
    in_maps = [{"x": xs[i]} for i in range(N_CORES)]
    trace = bool(int(os.environ.get("KERNEL_TRACE", "0")))
    LAST_RESULTS = run_bass_kernel_spmd(
        nc, in_maps, list(range(N_CORES)), trace=trace
    )
    out = np.stack(
        [LAST_RESULTS.results[i]["out"].reshape(1, H, W_IMG) for i in range(N_CORES)],
        axis=0,
    )
    return out.astype(np.float32, copy=False)


# revision 24
# speedup vs baseline: 100437.1055x; 100437.1055x over previous
"""Trainium2 Bass kernel for CurveChannel: piecewise-linear per-channel curve
+ 1x1 conv (C->1) + hardtanh(0,1).

out[b,0,h,w] = clip( sum_{p,c} W[p,c] * relu(x[b,c,h,w] - shift[p,c]) + conv_b,
                     0, 1 )         where W[p,c] = slopes[p,c] * conv_w[c]

Sharding: pure data parallel over batch (8 images -> 8 cores). Params are tiny
and get folded host-side into per-(p,c) weights; zero-weight terms contribute
exactly 0 and are skipped.

Per-core structure (memory-bound; ~4 MiB HBM traffic/core is the floor):
  - chunk the flat spatial dim; one combined HWDGE in-DMA per chunk
  - per nonzero term, a weighted relu into a slice of a per-chunk wide tile:
    ScalarE activation for most terms (W>0: W*relu(x-s) == relu(W*x - W*s);
    W<0: W*relu(x-s) == -relu(-W*x + W*s), subtracted later), with one
    shift==0 term offloaded to VectorE to balance engine load
  - VectorE combines slices (tensor-tensor adds for few terms, a strided
    tensor_reduce for many) and clips; per-chunk out-DMA
  - the last chunks are smaller to shorten the serial tail
"""

import os

import numpy as np

import concourse.bacc as bacc
import concourse.bass as bass
import concourse.mybir as mybir
import concourse.tile as tile
from concourse.bass_utils import run_bass_kernel_spmd

N_CORES = 8
C_IN = 3
H = 512
W_IMG = 512
P = 128                      # SBUF partitions
SPATIAL = H * W_IMG          # 262144
FREE = SPATIAL // P          # 2048 fp32 per partition per channel

# chunk schedule over the free dim (sums to FREE); smaller final chunks
# shorten the compute+store tail that cannot overlap the DMA stream
SCHEDULE = [256] * 7 + [128, 128]

F32 = mybir.dt.float32

LAST_RESULTS = None          # BassKernelResults of the most recent run (for test.py)


def _build_nc(terms, bias, reps=1, schedule=None, bufs=8, dve_offload=True,
              out_engine="sync"):
    """terms: list of (channel, weight, shift) with weight != 0.

    reps > 1 unrolls the whole pass multiple times over the same data --
    only used for benchmarking (marginal time per pass = device time with
    host/RPC constants cancelled).
    """
    schedule = list(schedule or SCHEDULE)
    assert sum(schedule) == FREE
    nc = bacc.Bacc(trn_type="TRN2", debug=False)
    x_t = nc.dram_tensor("x", [C_IN, P, FREE], F32, kind="ExternalInput")
    out_t = nc.dram_tensor("out", [P, FREE], F32, kind="ExternalOutput")

    pos = [(c, w, s) for c, w, s in terms if w > 0]
    neg = [(c, w, s) for c, w, s in terms if w < 0]
    # offload one positive shift==0 term to the vector engine (one
    # tensor_scalar: (x max 0) mult w) when ScalarE would otherwise have more
    # per-chunk work than VectorE; consumed last so the combine chain stays
    # same-engine
    dve_term = None
    if dve_offload and len(pos) + len(neg) >= 3:
        for i, (c, w, s) in enumerate(pos):
            if s == 0.0:
                dve_term = pos.pop(i)
                break
    ordered = pos + neg
    used_channels = sorted({c for c, _, _ in terms})
    cidx = {c: i for i, c in enumerate(used_channels)}
    nch = len(used_channels)
    nt = len(ordered)            # ACT-written slice count
    npos = len(pos)

    # activation float biases need pre-registered const APs (Bass only
    # registers 0.0/1.0); mirror Bass.__init__'s registration
    needed = set()
    for c, w, s in ordered:
        # keys must match the exact python float passed to activation()
        needed.add(float(-w * s) if w > 0 else float(w * s))
    for i, v in enumerate(sorted(needed)):
        if (F32, v) in nc.const_aps.aps:
            continue
        t = nc.alloc_sbuf_tensor(f"const-user-{i}", [P, 1], F32)
        nc.gpsimd.memset(t.ap(), v)
        nc.const_aps.aps[(F32, v)] = t.ap()
    if needed:
        nc.all_engine_barrier()

    with tile.TileContext(nc) as tc:
        with (
            tc.tile_pool(name="xin", bufs=bufs) as xpool,
            tc.tile_pool(name="work", bufs=bufs) as wpool,
            tc.tile_pool(name="out", bufs=bufs) as opool,
        ):
          for _ in range(reps):
            off = 0
            for CH in schedule:
                cs = slice(off, off + CH)
                off += CH
                res = opool.tile([P, CH], F32, tag="res")
                if nt == 0 and dve_term is None:
                    nc.vector.memset(res[:], float(np.clip(bias, 0.0, 1.0)))
                    nc.sync.dma_start(out=out_t[:, cs], in_=res[:])
                    continue

                xt = xpool.tile([P, nch * CH], F32, tag="x")
                if nch == C_IN:
                    nc.sync.dma_start(
                        out=xt[:],
                        in_=x_t[:, :, cs].rearrange("c p f -> p c f"),
                    )
                else:
                    for c in used_channels:
                        nc.sync.dma_start(
                            out=xt[:, bass.ts(cidx[c], CH)],
                            in_=x_t[c, :, cs],
                        )

                nslices = nt + (1 if dve_term is not None else 0)
                wide = wpool.tile([P, nslices * CH], F32, tag="wide")
                for i, (c, w, s) in enumerate(ordered):
                    sl = wide[:, bass.ts(i, CH)]
                    xs = xt[:, bass.ts(cidx[c], CH)]
                    if w > 0:
                        nc.scalar.activation(
                            sl, xs, mybir.ActivationFunctionType.Relu,
                            bias=-w * s, scale=w,
                        )
                    else:
                        nc.scalar.activation(
                            sl, xs, mybir.ActivationFunctionType.Relu,
                            bias=w * s, scale=-w,
                        )
                if dve_term is not None:
                    c, w, s = dve_term
                    nc.vector.tensor_scalar(
                        wide[:, bass.ts(nslices - 1, CH)],
                        xt[:, bass.ts(cidx[c], CH)],
                        0.0, w, mybir.AluOpType.max, mybir.AluOpType.mult,
                    )

                def combine(idxs, tag):
                    """sum of the given wide slices -> AP (None if empty)"""
                    if not idxs:
                        return None
                    if len(idxs) == 1:
                        return wide[:, bass.ts(idxs[0], CH)]
                    if len(idxs) <= 4 and idxs == list(
                        range(idxs[0], idxs[0] + len(idxs))
                    ):
                        acc = wpool.tile([P, CH], F32, tag=tag)
                        nc.vector.tensor_add(
                            acc[:], wide[:, bass.ts(idxs[0], CH)],
                            wide[:, bass.ts(idxs[1], CH)],
                        )
                        for k in idxs[2:]:
                            nc.vector.tensor_add(
                                acc[:], acc[:], wide[:, bass.ts(k, CH)]
                            )
                        return acc[:]
                    lo, hi = idxs[0], idxs[-1] + 1
                    dst = wpool.tile([P, CH], F32, tag=tag)
                    v = wide[:, lo * CH:hi * CH].rearrange(
                        "p (c f) -> p f c", c=hi - lo
                    )
                    nc.vector.tensor_reduce(
                        dst[:], v, axis=mybir.AxisListType.X,
                        op=mybir.AluOpType.add,
                    )
                    return dst[:]

                pos_idx = list(range(npos)) + (
                    [nslices - 1] if dve_term is not None else []
                )
                # keep the DVE slice in the positive combine only via the add
                # chain (it's not contiguous with the ACT positive slices)
                if dve_term is not None and npos >= 1:
                    rp_part = combine(list(range(npos)), "redp")
                    acc = wpool.tile([P, CH], F32, tag="accp")
                    nc.vector.tensor_add(
                        acc[:], rp_part, wide[:, bass.ts(nslices - 1, CH)]
                    )
                    rp = acc[:]
                elif dve_term is not None:
                    rp = wide[:, bass.ts(nslices - 1, CH)]
                else:
                    rp = combine(list(range(npos)), "redp")
                rn = combine(list(range(npos, nt)), "redn")

                if rp is not None and rn is not None:
                    comb = wpool.tile([P, CH], F32, tag="comb")
                    nc.vector.tensor_sub(comb[:], rp, rn)
                    comb = comb[:]
                elif rp is not None:
                    comb = rp
                else:
                    comb = wpool.tile([P, CH], F32, tag="comb")
                    nc.vector.tensor_scalar_mul(comb, rn, -1.0)
                    comb = comb[:]

                if bias != 0.0:
                    nc.vector.tensor_scalar(
                        res[:], comb, bias, 0.0,
                        mybir.AluOpType.add, mybir.AluOpType.max,
                    )
                    nc.vector.tensor_scalar_min(res[:], res[:], 1.0)
                else:
                    nc.vector.tensor_scalar(
                        res[:], comb, 0.0, 1.0,
                        mybir.AluOpType.max, mybir.AluOpType.min,
                    )
                oeng = nc.sync if out_engine == "sync" else nc.gpsimd
                oeng.dma_start(out=out_t[:, cs], in_=res[:])
    nc.compile()
    return nc


LINEAR_SCHEDULE = [512, 640, 512, 384]


def _build_linear_nc(w_common, bias, clip_mode, reps=1, schedule=None):
    """Raw-bacc fast path: out = clip(w_common*(x0+x1+x2) + bias, 0, 1) with
    every relu a no-op for the concrete input. Per chunk: 3 per-channel
    in-DMAs, two tensor_adds, one or two tensor_scalars, out-DMA. The first
    add is gated only on channels 0+1 so VectorE starts one DMA earlier.

    clip_mode "fused": bias==0, w>=0, x>=0 -- the lower clip is a no-op by
    f32 nonneg closure and the upper clip folds into the scale op
    ((sum mult w) min 1), which is exact. Otherwise the full two-op clip.
    """
    import contextlib
    schedule = list(schedule or LINEAR_SCHEDULE)
    assert sum(schedule) == FREE
    n = len(schedule)
    nc = bacc.Bacc(trn_type="TRN2", debug=False)
    x_t = nc.dram_tensor("x", [C_IN, P, FREE], F32, kind="ExternalInput")
    out_t = nc.dram_tensor("out", [P, FREE], F32, kind="ExternalOutput")
    xts = [nc.alloc_sbuf_tensor(f"xt{j}", [P, C_IN * CH], F32)
           for j, CH in enumerate(schedule)]
    tmps = [nc.alloc_sbuf_tensor(f"tmp{j}", [P, CH], F32)
            for j, CH in enumerate(schedule)]
    ress = [nc.alloc_sbuf_tensor(f"res{j}", [P, CH], F32)
            for j, CH in enumerate(schedule)]
    offs = np.cumsum([0] + schedule)
    with contextlib.ExitStack() as ctx:
        inA = [ctx.enter_context(nc.semaphore(f"inA{j}")) for j in range(n)]
        inB = [ctx.enter_context(nc.semaphore(f"inB{j}")) for j in range(n)]
        s1 = ctx.enter_context(nc.semaphore("s1"))
        s2 = ctx.enter_context(nc.semaphore("s2"))
        s3 = ctx.enter_context(nc.semaphore("s3"))
        dve_sem = ctx.enter_context(nc.semaphore("dve_sem"))
        out_sems = [ctx.enter_context(nc.semaphore(f"out{j}")) for j in range(n)]
        block = ctx.enter_context(nc.Block())

        @block.sync
        def _(sync):
            for r in range(reps):
                for j, CH in enumerate(schedule):
                    cs = slice(int(offs[j]), int(offs[j]) + CH)
                    if r > 0:
                        # WAR: previous rep's TT2 must have consumed xt{j}
                        sync.wait_ge(s2, (r - 1) * n + j + 1)
                    sync.dma_start(out=xts[j].ap()[:, bass.ts(0, CH)],
                                   in_=x_t[0, :, cs]).then_inc(inA[j], 16)
                    sync.dma_start(out=xts[j].ap()[:, bass.ts(1, CH)],
                                   in_=x_t[1, :, cs]).then_inc(inA[j], 16)
                    sync.dma_start(out=xts[j].ap()[:, bass.ts(2, CH)],
                                   in_=x_t[2, :, cs]).then_inc(inB[j], 16)
                for j, CH in enumerate(schedule):
                    cs = slice(int(offs[j]), int(offs[j]) + CH)
                    sync.wait_ge(dve_sem, r * n + j + 1)
                    sync.dma_start(out=out_t[:, cs],
                                   in_=ress[j].ap()).then_inc(out_sems[j], 16)
            for j in range(n):
                sync.wait_ge(out_sems[j], 16 * reps)

        @block.vector
        def _(vector):
            for r in range(reps):
                for j, CH in enumerate(schedule):
                    xa = xts[j].ap()
                    k = r * n + j + 1
                    vector.wait_ge(inA[j], 32 * (r + 1))
                    vector.tensor_add(
                        tmps[j].ap(), xa[:, bass.ts(0, CH)],
                        xa[:, bass.ts(1, CH)],
                    ).then_inc(s1, 1)
                    vector.wait_ge(inB[j], 16 * (r + 1))
                    vector.wait_ge(s1, k)
                    vector.tensor_add(
                        tmps[j].ap(), tmps[j].ap(), xa[:, bass.ts(2, CH)]
                    ).then_inc(s2, 1)
                    vector.wait_ge(s2, k)
                    if r > 0:
                        # WAR: previous rep's out-DMA must have read res{j}
                        vector.wait_ge(out_sems[j], 16 * r)
                    if clip_mode == "fused":
                        vector.tensor_scalar(
                            ress[j].ap(), tmps[j].ap(), w_common, 1.0,
                            mybir.AluOpType.mult, mybir.AluOpType.min,
                        ).then_inc(dve_sem, 1)
                    else:
                        vector.tensor_scalar(
                            ress[j].ap(), tmps[j].ap(), w_common, bias,
                            mybir.AluOpType.mult, mybir.AluOpType.add,
                        ).then_inc(s3, 1)
                        vector.wait_ge(s3, k)
                        vector.tensor_scalar(
                            ress[j].ap(), ress[j].ap(), 0.0, 1.0,
                            mybir.AluOpType.max, mybir.AluOpType.min,
                        ).then_inc(dve_sem, 1)
    nc.compile()
    return nc


_NC_CACHE = {}


def _fast_linear_plan(terms, bias, xmin):
    """If every relu is a no-op for the concrete input (all shifts <= xmin),
    the model is linear: out = clip(sum_c Wc*x_c + b', 0, 1) with
    Wc = sum_p w[p,c], b' = bias - sum w*s. Returns (w_common, b', clip_mode)
    when additionally all Wc are equal (single post-scale), else None."""
    if not terms:
        return None
    if any(s > xmin for _, _, s in terms):
        return None
    bprime = bias - sum(w * s for _, w, s in terms)
    wc = {}
    for c, w, s in terms:
        wc[c] = wc.get(c, 0.0) + w
    if set(wc) != set(range(C_IN)):
        return None
    vals = list(wc.values())
    if max(vals) != min(vals):
        return None
    w_common = vals[0]
    if bprime == 0.0 and w_common >= 0.0 and xmin >= 0.0:
        clip_mode = "fused"      # exact: see _build_linear_nc
    else:
        clip_mode = "full"
    return (w_common, bprime, clip_mode)


def kernel(x, shift, slopes, conv_w, conv_b):
    global LAST_RESULTS
    x = np.ascontiguousarray(np.asarray(x, dtype=np.float32))
    shift = np.asarray(shift, dtype=np.float32)
    slopes = np.asarray(slopes, dtype=np.float32)
    conv_w = np.asarray(conv_w, dtype=np.float32)
    conv_b = np.asarray(conv_b, dtype=np.float32)

    B = x.shape[0]
    assert x.shape == (N_CORES, C_IN, H, W_IMG), x.shape

    wmat = slopes * conv_w[None, :]                      # (npts, C)
    npts = wmat.shape[0]
    terms = tuple(
        (c, float(wmat[p, c]), float(shift[p, c]))
        for p in range(npts) for c in range(C_IN)
        if wmat[p, c] != 0.0
    )
    bias = float(conv_b.reshape(-1)[0])

    xmin = float(x.min())
    plan = _fast_linear_plan(terms, bias, xmin)
    if plan is not None:
        w_common, bprime, clip_mode = plan
        key = ("lin", w_common, bprime, clip_mode)
        nc = _NC_CACHE.get(key)
        if nc is None:
            nc = _build_linear_nc(w_common, bprime, clip_mode)
            _NC_CACHE[key] = nc
    else:
        key = (terms, bias)
        nc = _NC_CACHE.get(key)
        if nc is None:
            nc = _build_nc(terms, bias)
            _NC_CACHE[key] = nc

    xs = x.reshape(B, C_IN, P, FREE)
    in_maps = [{"x": xs[i]} for i in range(N_CORES)]
    trace = bool(int(os.environ.get("KERNEL_TRACE", "0")))
    LAST_RESULTS = run_bass_kernel_spmd(
        nc, in_maps, list(range(N_CORES)), trace=trace
    )
    out = np.stack(
        [LAST_RESULTS.results[i]["out"].reshape(1, H, W_IMG) for i in range(N_CORES)],
        axis=0,
    )
    return out.astype(np.float32, copy=False)
